# revision 29
# baseline (speedup 1.0000x reference)
"""Trainium2 Bass kernel for ContextQueryAttention (BiDAF-style trilinear
attention). Data-parallel over batch across 8 NeuronCores (4 batches/core).

Per batch (c=1024 context rows, q=128 query rows, h=256 hidden):
  S[c,q]   = ctx@cw + (qry@qw)^T + (ctx*cqw)@qry^T + bias
  S_bar    = softmax_c(S); S_bar_bar = softmax_q(S)
  A        = S @ qry
  B        = S_bar @ (S_bar_bar^T @ ctx)
  out      = concat([ctx, A, ctx*A, ctx*B], -1)

v5: all-bf16 I/O + 2-deep software pipelining.
  - ctx@cw (s0) folds into the S^T contraction (lhsT = qry^T*cqw + cw);
    s1+bias rides one K=1 rank-1 matmul per 512-col chunk.
  - One exp pass (fused row-sums) serves both softmaxes: 1/Zc folds into
    the transposed exp tiles, 1/Zq into T.
  - T(b) and B(b) matmuls are deferred one iteration: the per-tile
    zc -> 1/zc -> e-scale chain of batch b gets a full cycle of slack, so
    every PE phase in the steady state has pre-satisfied inputs:
      PE/iter b: S^T(b), T(b-1), A(b), transposes(b), B(b-1).
  - Evac split: exp/A-copy/es-odd on ACT; straws/ts/ctxB/zc/es-even on
    DVE; zq/ctx*A on Pool (SBUF-only engine; ctx*A reads the copied
    SBUF A-channel).
  - Outputs bf16, ctx passthrough channel assembled host-side.
"""

import numpy as np

B, C, Q, H = 32, 1024, 128, 256
N_CORES = 8
BPC = B // N_CORES  # batches per core
P = 128
HC = H // P  # h chunks of 128
CT = C // P  # c tiles of 128
CCH = 512  # S^T free-dim chunk (1 PSUM bank of fp32)
NCC = C // CCH

_NC_CACHE = {}


def _build_kernel():
    import concourse.bacc as bacc
    import concourse.tile as tile
    from concourse import mybir
    from concourse.masks import make_identity

    f32 = mybir.dt.float32
    bf16 = mybir.dt.bfloat16
    AF = mybir.ActivationFunctionType
    AX = mybir.AxisListType
    ALU = mybir.AluOpType

    nc = bacc.Bacc(trn_type="TRN2", target_bir_lowering=False, debug=False)
    ctx_d = nc.dram_tensor("ctx", [BPC, P, CT * H], bf16, kind="ExternalInput").ap()
    # chunked [b, cc, p, j*cch] so each 512-col S^T chunk arrives separately
    ctxT_d = nc.dram_tensor(
        "ctxT", [BPC, NCC, P, HC * CCH], bf16, kind="ExternalInput"
    ).ap()
    # packed bf16 consts: [0:1024]=qry^T, [1024:2048]=qry, [2048:2050]=qw cols
    qb_d = nc.dram_tensor("qb", [P, 2 * HC * BPC * Q + HC], bf16, kind="ExternalInput").ap()
    # packed f32 consts: [0:HC]=cqw cols, [HC:2HC]=cw cols, [2HC]=bias
    wv_d = nc.dram_tensor("wv", [P, 2 * HC + 1], f32, kind="ExternalInput").ap()
    out_d = nc.dram_tensor("out", [BPC, P, CT * 3 * H], bf16, kind="ExternalOutput").ap()

    from contextlib import ExitStack

    with tile.TileContext(nc) as tc, ExitStack() as es:
        consts = es.enter_context(tc.tile_pool(name="consts", bufs=1))
        p_et = es.enter_context(tc.tile_pool(name="p_et", bufs=2))
        p_sr = es.enter_context(tc.tile_pool(name="p_sr", bufs=2))
        p_esb = es.enter_context(tc.tile_pool(name="p_esb", bufs=2))
        p_out = es.enter_context(tc.tile_pool(name="p_out", bufs=3))
        p_vec = es.enter_context(tc.tile_pool(name="p_vec", bufs=2))
        pp_st = es.enter_context(tc.tile_pool(name="pp_st", bufs=2, space="PSUM"))
        pp_tr = es.enter_context(tc.tile_pool(name="pp_tr", bufs=1, space="PSUM"))
        pp_t = es.enter_context(tc.tile_pool(name="pp_t", bufs=1, space="PSUM"))
        pp_a = es.enter_context(tc.tile_pool(name="pp_a", bufs=2, space="PSUM"))
        pp_b = es.enter_context(tc.tile_pool(name="pp_b", bufs=2, space="PSUM"))

        # ---- const DMAs (2 packed transfers) ----
        wv = consts.tile([P, 2 * HC + 1], f32)
        nc.sync.dma_start(out=wv, in_=wv_d)
        qb = consts.tile([P, 2 * HC * BPC * Q + HC], bf16)
        nc.sync.dma_start(out=qb, in_=qb_d)
        qt_all = qb[:, 0 : HC * BPC * Q].rearrange("p (j bq) -> p j bq", j=HC)
        qa_all = qb[:, HC * BPC * Q : 2 * HC * BPC * Q].rearrange(
            "p (b h) -> p b h", b=BPC
        )
        qwr = qb[:, 2 * HC * BPC * Q :]
        bias_sb = wv[0:1, 2 * HC : 2 * HC + 1]

        identity = consts.tile([P, P], bf16)
        make_identity(nc, identity)
        ones_c = consts.tile([1, CCH], bf16)
        nc.vector.memset(ones_c, 1.0)

        # ---- big input DMAs for first two batches (batch 0 via the ACT
        # queue so it transfers concurrently with the SP-queue consts) ----
        ctxT_all = consts.tile([P, BPC, NCC, HC, CCH], bf16)
        ctx_all = consts.tile([P, BPC, CT, H], bf16)
        for cc in range(NCC):
            nc.scalar.dma_start(
                out=ctxT_all[:, 0, cc].rearrange("p j c -> p (j c)"),
                in_=ctxT_d[0, cc],
            )
        nc.scalar.dma_start(
            out=ctx_all[:, 0].rearrange("p t h -> p (t h)"), in_=ctx_d[0]
        )
        for cc in range(NCC):
            nc.sync.dma_start(
                out=ctxT_all[:, 1, cc].rearrange("p j c -> p (j c)"),
                in_=ctxT_d[1, cc],
            )
        nc.sync.dma_start(
            out=ctx_all[:, 1].rearrange("p t h -> p (t h)"), in_=ctx_d[1]
        )

        # ---- preamble: qt_cq = qry^T*cqw + cw (folds s0) on DVE; s1 rows ----
        qt_cq = consts.tile([P, HC, BPC * Q], bf16)
        for j in range(HC):
            nc.vector.tensor_scalar(
                qt_cq[:, j],
                qt_all[:, j],
                wv[:, j : j + 1],
                wv[:, HC + j : HC + j + 1],
                ALU.mult,
                ALU.add,
            )
        s1p = pp_st.tile([1, BPC * Q], f32, tag="stp")
        for j in range(HC):
            nc.tensor.matmul(
                s1p,
                lhsT=qwr[:, j : j + 1],
                rhs=qt_all[:, j],
                start=(j == 0),
                stop=(j == HC - 1),
            )
        s1_rows = consts.tile([1, BPC * Q], bf16)
        nc.scalar.activation(s1_rows, s1p, AF.Identity, bias=bias_sb, scale=1.0)

        # cross-iteration state of batch b-1: (b, e_t, e_sb, rq, out_t)
        prev = None

        def emit_t_phase(state):
            """T(b-1) = S_bar_bar^T @ ctx; ts = T * rq.  Returns ts."""
            bp, e_tp, e_sbp, rqp, _ = state
            t_acc = pp_t.tile([P, H], f32, tag="t_acc", name=f"tacc{bp}")
            for t in range(CT):
                nc.tensor.matmul(
                    t_acc,
                    lhsT=e_sbp[:, t, :],
                    rhs=ctx_all[:, bp, t, :],
                    start=(t == 0),
                    stop=(t == CT - 1),
                )
            ts = p_vec.tile([P, H], bf16, tag="ts", name=f"ts{bp}")
            nc.vector.tensor_scalar_mul(ts, t_acc, rqp)
            return ts

        def emit_b_pair(state, ts, p2):
            """One B(b-1) c-tile pair + its ctx*B on DVE."""
            bp, e_tp, _, _, out_tp = state
            t0 = 2 * p2
            pb = pp_b.tile([P, 2, H], f32, tag="pb", name=f"pb{bp}{p2}")
            for k in range(2):
                nc.tensor.matmul(
                    pb[:, k, :],
                    lhsT=e_tp[:, (t0 + k) * P : (t0 + k + 1) * P],
                    rhs=ts,
                    start=True,
                    stop=True,
                )
            nc.vector.tensor_mul(
                out_tp[:, t0 : t0 + 2, 2 * H : 3 * H],
                ctx_all[:, bp, t0 : t0 + 2, :],
                pb,
            )

        def emit_stores(state):
            bp, _, _, _, out_tp = state
            nc.sync.dma_start(
                out=out_d[bp, :, 0 : 4 * 3 * H],
                in_=out_tp[:, 0:4, :].rearrange("p t h3 -> p (t h3)"),
            )
            nc.sync.dma_start(
                out=out_d[bp, :, 4 * 3 * H :],
                in_=out_tp[:, 4:CT, :].rearrange("p t h3 -> p (t h3)"),
            )

        for b in range(BPC):
            if b + 2 < BPC:
                for cc in range(NCC):
                    nc.sync.dma_start(
                        out=ctxT_all[:, b + 2, cc].rearrange("p j c -> p (j c)"),
                        in_=ctxT_d[b + 2, cc],
                    )
                nc.sync.dma_start(
                    out=ctx_all[:, b + 2].rearrange("p t h -> p (t h)"),
                    in_=ctx_d[b + 2],
                )

            bq = slice(b * Q, (b + 1) * Q)

            # ---- S^T [q, c] (incl s0 via qt_cq, s1+bias via rider); exp ----
            e_t = p_et.tile([P, C], bf16, tag="e_t")
            st_raw = p_sr.tile([P, C], bf16, tag="st_raw")
            rsum = p_vec.tile([P, NCC], f32, tag="rsum")
            for cc in range(NCC):
                sl = slice(cc * CCH, (cc + 1) * CCH)
                stp = pp_st.tile([P, CCH], f32, tag="stp")
                for j in range(HC):
                    nc.tensor.matmul(
                        stp,
                        lhsT=qt_cq[:, j, bq],
                        rhs=ctxT_all[:, b, cc, j, :],
                        start=(j == 0),
                        stop=False,
                    )
                nc.tensor.matmul(
                    stp, lhsT=s1_rows[0:1, bq], rhs=ones_c, start=False, stop=True
                )
                nc.scalar.activation(
                    e_t[:, sl], stp, AF.Exp, accum_out=rsum[:, cc : cc + 1]
                )
                nc.vector.tensor_copy(st_raw[:, sl], stp)

            # softmax_c denominators (zq on Pool, reciprocal on DVE)
            zq = p_vec.tile([P, 1], f32, tag="zq")
            nc.gpsimd.tensor_add(zq, rsum[:, 0:1], rsum[:, 1:2])
            rq = p_vec.tile([P, 1], f32, tag="rq")
            nc.vector.reciprocal(rq, zq)

            # ---- deferred T-phase of batch b-1 (its e_sb is long ready) ----
            ts_prev = emit_t_phase(prev) if prev is not None else None

            out_t = p_out.tile([P, CT, 3 * H], bf16, tag="out_t")

            # ---- A(b) pairs interleaved with B(b-1) pairs: each PSUM pool's
            # allocations are spaced a full pair apart, so the ACT A-copy and
            # DVE ctx*B evacuations run in parallel and keep up with the PE --
            for p2 in range(CT // 2):
                t0 = 2 * p2
                pa = pp_a.tile([P, 2, H], f32, tag="pa", name=f"pa{b}{p2}")
                for k in range(2):
                    nc.tensor.matmul(
                        pa[:, k, :],
                        lhsT=st_raw[:, (t0 + k) * P : (t0 + k + 1) * P],
                        rhs=qa_all[:, b, :],
                        start=True,
                        stop=True,
                    )
                nc.scalar.copy(out_t[:, t0 : t0 + 2, 0:H], pa)
                nc.gpsimd.tensor_mul(
                    out_t[:, t0 : t0 + 2, H : 2 * H],
                    ctx_all[:, b, t0 : t0 + 2, :],
                    out_t[:, t0 : t0 + 2, 0:H],
                )
                if prev is not None:
                    emit_b_pair(prev, ts_prev, p2)

            # ---- E-transposes; zc/rc/e-scale (consumed only next iter) ----
            tr8 = pp_tr.tile([P, CT, P], bf16, tag="tr8")
            for t in range(CT):
                nc.tensor.transpose(tr8[:, t, :], e_t[:, t * P : (t + 1) * P], identity)
            e_sb = p_esb.tile([P, CT, P], bf16, tag="e_sb")
            zc8 = p_vec.tile([P, CT], f32, tag="zc8")
            rc8 = p_vec.tile([P, CT], f32, tag="rc8")
            nc.vector.reduce_sum(zc8, tr8, axis=AX.X)
            nc.vector.reciprocal(rc8, zc8)
            for t in range(CT):
                if t % 2 == 0:
                    nc.vector.tensor_scalar_mul(
                        e_sb[:, t, :], tr8[:, t, :], rc8[:, t : t + 1]
                    )
                else:
                    nc.scalar.mul(e_sb[:, t, :], tr8[:, t, :], rc8[:, t : t + 1])

            # ---- store of batch b-1 ----
            if prev is not None:
                emit_stores(prev)

            prev = (b, e_t, e_sb, rq, out_t)

        # epilogue: T/B/store for the last batch
        ts_last = emit_t_phase(prev)
        for p2 in range(CT // 2):
            emit_b_pair(prev, ts_last, p2)
        emit_stores(prev)

    nc.compile()
    return nc


def _get_nc():
    if "nc" not in _NC_CACHE:
        _NC_CACHE["nc"] = _build_kernel()
    return _NC_CACHE["nc"]


def make_in_maps(context, query, c_weight, q_weight, cq_weight, bias):
    import ml_dtypes

    bf16 = ml_dtypes.bfloat16
    context = np.ascontiguousarray(np.asarray(context, dtype=np.float32))
    query = np.asarray(query, dtype=np.float32)
    cw = np.asarray(c_weight, dtype=np.float32).reshape(H)
    qw = np.asarray(q_weight, dtype=np.float32).reshape(H)
    cqw = np.asarray(cq_weight, dtype=np.float32).reshape(H)
    bs = float(np.asarray(bias, dtype=np.float32).reshape(1)[0])

    # [:, 0:HC]=cqw cols, [:, HC:2HC]=cw cols, [:, 2HC]=bias (col j is h=j*128+p)
    wv = np.concatenate(
        [
            cqw.reshape(HC, P).T,
            cw.reshape(HC, P).T,
            np.full((P, 1), bs, np.float32),
        ],
        axis=1,
    ).astype(np.float32)
    wv = np.ascontiguousarray(wv)
    qwr = qw.reshape(HC, P).T.astype(bf16)

    in_maps = []
    for i in range(N_CORES):
        sl = slice(i * BPC, (i + 1) * BPC)
        ctx_i = context[sl]
        qry_i = query[sl]
        # ctx: [b, c, h] -> [b, p, t, h] with c = t*128+p
        ctx_s = np.ascontiguousarray(
            ctx_i.reshape(BPC, CT, P, H).transpose(0, 2, 1, 3).reshape(BPC, P, CT * H)
        ).astype(bf16)
        # ctxT: [b, h, c] -> [b, cc, p, j, cch] with h = j*128+p, c = cc*512+cch
        ctxT_s = np.ascontiguousarray(
            ctx_i.transpose(0, 2, 1)
            .reshape(BPC, HC, P, NCC, CCH)
            .transpose(0, 3, 2, 1, 4)
            .reshape(BPC, NCC, P, HC * CCH)
        ).astype(bf16)
        # qry^T: [b, h, q] -> [p, j, b, q]
        qt_s = (
            qry_i.transpose(0, 2, 1)
            .reshape(BPC, HC, P, Q)
            .transpose(2, 1, 0, 3)
            .reshape(P, HC * BPC * Q)
        ).astype(bf16)
        # qry: [b, q, h] -> [q, b, h]
        qa_s = qry_i.transpose(1, 0, 2).reshape(P, BPC * H).astype(bf16)
        qb = np.ascontiguousarray(np.concatenate([qt_s, qa_s, qwr], axis=1))
        in_maps.append({"ctx": ctx_s, "ctxT": ctxT_s, "qb": qb, "wv": wv})
    return in_maps


def kernel(context, query, c_mask, q_mask, c_weight, q_weight, cq_weight, bias):
    from concourse import bass_utils

    nc = _get_nc()
    in_maps = make_in_maps(context, query, c_weight, q_weight, cq_weight, bias)
    res = bass_utils.run_bass_kernel_spmd(nc, in_maps, core_ids=list(range(N_CORES)))

    context = np.asarray(context, dtype=np.float32)
    full = np.empty((B, C, 4 * H), dtype=np.float32)
    full[:, :, 0:H] = context
    for i in range(N_CORES):
        # device out: [b, p, t, 3h] -> [b, (t p), 3h]
        o = res.results[i]["out"].reshape(BPC, P, CT, 3 * H).transpose(0, 2, 1, 3)
        full[i * BPC : (i + 1) * BPC, :, H:] = o.reshape(BPC, C, 3 * H).astype(
            np.float32
        )
    return full


# revision 31
# speedup vs baseline: 1.1631x; 1.1631x over previous
"""Trainium2 Bass kernel for ContextQueryAttention (BiDAF-style trilinear
attention). Data-parallel over batch across 8 NeuronCores (4 batches/core).

Per batch (c=1024 context rows, q=128 query rows, h=256 hidden):
  S[c,q]   = ctx@cw + (qry@qw)^T + (ctx*cqw)@qry^T + bias
  S_bar    = softmax_c(S); S_bar_bar = softmax_q(S)
  A        = S @ qry
  B        = S_bar @ (S_bar_bar^T @ ctx)
  out      = concat([ctx, A, ctx*A, ctx*B], -1)

v5: all-bf16 I/O + 2-deep software pipelining.
  - ctx@cw (s0) folds into the S^T contraction (lhsT = qry^T*cqw + cw);
    s1+bias rides one K=1 rank-1 matmul per 512-col chunk.
  - One exp pass (fused row-sums) serves both softmaxes: 1/Zc folds into
    the transposed exp tiles, 1/Zq into T.
  - T(b) and B(b) matmuls are deferred one iteration: the per-tile
    zc -> 1/zc -> e-scale chain of batch b gets a full cycle of slack, so
    every PE phase in the steady state has pre-satisfied inputs:
      PE/iter b: S^T(b), T(b-1), A(b), transposes(b), B(b-1).
  - Evac split: exp/A-copy/es-odd on ACT; straws/ts/ctxB/zc/es-even on
    DVE; zq/ctx*A on Pool (SBUF-only engine; ctx*A reads the copied
    SBUF A-channel).
  - Outputs bf16, ctx passthrough channel assembled host-side.
"""

import numpy as np

B, C, Q, H = 32, 1024, 128, 256
N_CORES = 8
BPC = B // N_CORES  # batches per core
P = 128
HC = H // P  # h chunks of 128
CT = C // P  # c tiles of 128
CCH = 512  # S^T free-dim chunk (1 PSUM bank of fp32)
NCC = C // CCH

_NC_CACHE = {}


def _build_kernel():
    import concourse.bacc as bacc
    import concourse.tile as tile
    from concourse import mybir
    from concourse.masks import make_identity

    f32 = mybir.dt.float32
    bf16 = mybir.dt.bfloat16
    AF = mybir.ActivationFunctionType
    AX = mybir.AxisListType
    ALU = mybir.AluOpType

    nc = bacc.Bacc(trn_type="TRN2", target_bir_lowering=False, debug=False)
    ctx_d = nc.dram_tensor("ctx", [BPC, P, CT * H], bf16, kind="ExternalInput").ap()
    # chunked [b, cc, p, j*cch] so each 512-col S^T chunk arrives separately
    ctxT_d = nc.dram_tensor(
        "ctxT", [BPC, NCC, P, HC * CCH], bf16, kind="ExternalInput"
    ).ap()
    # packed bf16 consts: qtw = qry^T + qw cols (preamble-critical), qa = qry
    qtw_d = nc.dram_tensor("qtw", [P, HC * BPC * Q + HC], bf16, kind="ExternalInput").ap()
    qa_d = nc.dram_tensor("qa", [P, BPC * H], bf16, kind="ExternalInput").ap()
    # packed f32 consts: [0:HC]=cqw cols, [HC:2HC]=cw cols, [2HC]=bias
    wv_d = nc.dram_tensor("wv", [P, 2 * HC + 1], f32, kind="ExternalInput").ap()
    out_d = nc.dram_tensor("out", [BPC, P, CT * 3 * H], bf16, kind="ExternalOutput").ap()

    from contextlib import ExitStack

    with tile.TileContext(nc) as tc, ExitStack() as es:
        consts = es.enter_context(tc.tile_pool(name="consts", bufs=1))
        p_et = es.enter_context(tc.tile_pool(name="p_et", bufs=2))
        p_sr = es.enter_context(tc.tile_pool(name="p_sr", bufs=2))
        p_esb = es.enter_context(tc.tile_pool(name="p_esb", bufs=2))
        p_out = es.enter_context(tc.tile_pool(name="p_out", bufs=3))
        p_vec = es.enter_context(tc.tile_pool(name="p_vec", bufs=2))
        pp_st = es.enter_context(tc.tile_pool(name="pp_st", bufs=2, space="PSUM"))
        pp_tr = es.enter_context(tc.tile_pool(name="pp_tr", bufs=1, space="PSUM"))
        pp_t = es.enter_context(tc.tile_pool(name="pp_t", bufs=1, space="PSUM"))
        pp_a = es.enter_context(tc.tile_pool(name="pp_a", bufs=2, space="PSUM"))
        pp_b = es.enter_context(tc.tile_pool(name="pp_b", bufs=2, space="PSUM"))

        # ---- const DMAs (critical qtw first, then wv, then qa) ----
        qtw = consts.tile([P, HC * BPC * Q + HC], bf16)
        nc.sync.dma_start(out=qtw, in_=qtw_d)
        wv = consts.tile([P, 2 * HC + 1], f32)
        nc.sync.dma_start(out=wv, in_=wv_d)
        qa_sb = consts.tile([P, BPC * H], bf16)
        nc.sync.dma_start(out=qa_sb, in_=qa_d)
        qt_all = qtw[:, 0 : HC * BPC * Q].rearrange("p (j bq) -> p j bq", j=HC)
        qa_all = qa_sb.rearrange("p (b h) -> p b h", b=BPC)
        qwr = qtw[:, HC * BPC * Q :]
        bias_sb = wv[0:1, 2 * HC : 2 * HC + 1]

        identity = consts.tile([P, P], bf16)
        make_identity(nc, identity)
        ones_c = consts.tile([1, CCH], bf16)
        nc.vector.memset(ones_c, 1.0)

        # ---- big input DMAs for first two batches (batch 0 via the ACT
        # queue so it transfers concurrently with the SP-queue consts) ----
        ctxT_all = consts.tile([P, BPC, NCC, HC, CCH], bf16)
        ctx_all = consts.tile([P, BPC, CT, H], bf16)
        for cc in range(NCC):
            nc.scalar.dma_start(
                out=ctxT_all[:, 0, cc].rearrange("p j c -> p (j c)"),
                in_=ctxT_d[0, cc],
            )
        nc.scalar.dma_start(
            out=ctx_all[:, 0].rearrange("p t h -> p (t h)"), in_=ctx_d[0]
        )
        for cc in range(NCC):
            nc.sync.dma_start(
                out=ctxT_all[:, 1, cc].rearrange("p j c -> p (j c)"),
                in_=ctxT_d[1, cc],
            )
        nc.sync.dma_start(
            out=ctx_all[:, 1].rearrange("p t h -> p (t h)"), in_=ctx_d[1]
        )

        # ---- preamble: qt_cq = qry^T*cqw + cw (folds s0) on DVE; s1 rows ----
        qt_cq = consts.tile([P, HC, BPC * Q], bf16)
        for j in range(HC):
            nc.vector.tensor_scalar(
                qt_cq[:, j],
                qt_all[:, j],
                wv[:, j : j + 1],
                wv[:, HC + j : HC + j + 1],
                ALU.mult,
                ALU.add,
            )
        s1p = pp_st.tile([1, BPC * Q], f32, tag="stp")
        for j in range(HC):
            nc.tensor.matmul(
                s1p,
                lhsT=qwr[:, j : j + 1],
                rhs=qt_all[:, j],
                start=(j == 0),
                stop=(j == HC - 1),
            )
        s1_rows = consts.tile([1, BPC * Q], bf16)
        nc.scalar.activation(s1_rows, s1p, AF.Identity, bias=bias_sb, scale=1.0)

        # cross-iteration state of batch b-1: (b, e_t, e_sb, rq, out_t)
        prev = None

        def emit_t_phase(state):
            """T(b-1) = S_bar_bar^T @ ctx; ts = T * rq.  Returns ts."""
            bp, e_tp, e_sbp, rqp, _ = state
            t_acc = pp_t.tile([P, H], f32, tag="t_acc", name=f"tacc{bp}")
            for t in range(CT):
                nc.tensor.matmul(
                    t_acc,
                    lhsT=e_sbp[:, t, :],
                    rhs=ctx_all[:, bp, t, :],
                    start=(t == 0),
                    stop=(t == CT - 1),
                )
            ts = p_vec.tile([P, H], bf16, tag="ts", name=f"ts{bp}")
            nc.vector.tensor_scalar_mul(ts, t_acc, rqp)
            return ts

        def emit_b_phase(state, ts):
            """B(b-1) pairs + ctx*B on DVE, then store batch b-1."""
            bp, e_tp, _, _, out_tp = state
            for p2 in range(CT // 2):
                t0 = 2 * p2
                pb = pp_b.tile([P, 2, H], f32, tag="pb", name=f"pb{bp}{p2}")
                for k in range(2):
                    nc.tensor.matmul(
                        pb[:, k, :],
                        lhsT=e_tp[:, (t0 + k) * P : (t0 + k + 1) * P],
                        rhs=ts,
                        start=True,
                        stop=True,
                    )
                nc.vector.tensor_mul(
                    out_tp[:, t0 : t0 + 2, 2 * H : 3 * H],
                    ctx_all[:, bp, t0 : t0 + 2, :],
                    pb,
                )
            nc.sync.dma_start(
                out=out_d[bp, :, 0 : 4 * 3 * H],
                in_=out_tp[:, 0:4, :].rearrange("p t h3 -> p (t h3)"),
            )
            nc.sync.dma_start(
                out=out_d[bp, :, 4 * 3 * H :],
                in_=out_tp[:, 4:CT, :].rearrange("p t h3 -> p (t h3)"),
            )

        for b in range(BPC):
            if b + 2 < BPC:
                for cc in range(NCC):
                    nc.sync.dma_start(
                        out=ctxT_all[:, b + 2, cc].rearrange("p j c -> p (j c)"),
                        in_=ctxT_d[b + 2, cc],
                    )
                nc.sync.dma_start(
                    out=ctx_all[:, b + 2].rearrange("p t h -> p (t h)"),
                    in_=ctx_d[b + 2],
                )

            bq = slice(b * Q, (b + 1) * Q)

            # ---- S^T [q, c] (incl s0 via qt_cq, s1+bias via rider); exp ----
            e_t = p_et.tile([P, C], bf16, tag="e_t")
            st_raw = p_sr.tile([P, C], bf16, tag="st_raw")
            rsum = p_vec.tile([P, NCC], f32, tag="rsum")
            for cc in range(NCC):
                sl = slice(cc * CCH, (cc + 1) * CCH)
                stp = pp_st.tile([P, CCH], f32, tag="stp")
                for j in range(HC):
                    nc.tensor.matmul(
                        stp,
                        lhsT=qt_cq[:, j, bq],
                        rhs=ctxT_all[:, b, cc, j, :],
                        start=(j == 0),
                        stop=False,
                    )
                nc.tensor.matmul(
                    stp, lhsT=s1_rows[0:1, bq], rhs=ones_c, start=False, stop=True
                )
                nc.scalar.activation(
                    e_t[:, sl], stp, AF.Exp, accum_out=rsum[:, cc : cc + 1]
                )
                nc.vector.tensor_copy(st_raw[:, sl], stp)

            # softmax_c denominators (zq on Pool, reciprocal on DVE)
            zq = p_vec.tile([P, 1], f32, tag="zq")
            nc.gpsimd.tensor_add(zq, rsum[:, 0:1], rsum[:, 1:2])
            rq = p_vec.tile([P, 1], f32, tag="rq")
            nc.vector.reciprocal(rq, zq)

            # ---- deferred T-phase of batch b-1 (its e_sb is long ready) ----
            ts_prev = emit_t_phase(prev) if prev is not None else None

            out_t = p_out.tile([P, CT, 3 * H], bf16, tag="out_t")

            # ---- A per c-tile pair; copy on ACT, ctx*A on Pool (SBUF) ----
            for p2 in range(CT // 2):
                t0 = 2 * p2
                pa = pp_a.tile([P, 2, H], f32, tag="pa", name=f"pa{b}{p2}")
                for k in range(2):
                    nc.tensor.matmul(
                        pa[:, k, :],
                        lhsT=st_raw[:, (t0 + k) * P : (t0 + k + 1) * P],
                        rhs=qa_all[:, b, :],
                        start=True,
                        stop=True,
                    )
                nc.scalar.copy(out_t[:, t0 : t0 + 2, 0:H], pa)
                ctxa_engine = nc.vector if b == BPC - 1 else nc.gpsimd
                ctxa_engine.tensor_mul(
                    out_t[:, t0 : t0 + 2, H : 2 * H],
                    ctx_all[:, b, t0 : t0 + 2, :],
                    out_t[:, t0 : t0 + 2, 0:H],
                )

            # ---- E-transposes; zc/rc/e-scale (consumed only next iter) ----
            tr8 = pp_tr.tile([P, CT, P], bf16, tag="tr8")
            for t in range(CT):
                nc.tensor.transpose(tr8[:, t, :], e_t[:, t * P : (t + 1) * P], identity)
            e_sb = p_esb.tile([P, CT, P], bf16, tag="e_sb")
            zc8 = p_vec.tile([P, CT], f32, tag="zc8")
            rc8 = p_vec.tile([P, CT], f32, tag="rc8")
            nc.vector.reduce_sum(zc8, tr8, axis=AX.X)
            nc.vector.reciprocal(rc8, zc8)
            for t in range(CT):
                if t % 2 == 0:
                    nc.vector.tensor_scalar_mul(
                        e_sb[:, t, :], tr8[:, t, :], rc8[:, t : t + 1]
                    )
                else:
                    nc.scalar.mul(e_sb[:, t, :], tr8[:, t, :], rc8[:, t : t + 1])

            # ---- deferred B-phase + store of batch b-1 ----
            if prev is not None:
                emit_b_phase(prev, ts_prev)

            prev = (b, e_t, e_sb, rq, out_t)

        # epilogue: T/B/store for the last batch
        ts_last = emit_t_phase(prev)
        emit_b_phase(prev, ts_last)

    nc.compile()
    return nc


def _get_nc():
    if "nc" not in _NC_CACHE:
        _NC_CACHE["nc"] = _build_kernel()
    return _NC_CACHE["nc"]


def make_in_maps(context, query, c_weight, q_weight, cq_weight, bias):
    import ml_dtypes

    bf16 = ml_dtypes.bfloat16
    context = np.ascontiguousarray(np.asarray(context, dtype=np.float32))
    query = np.asarray(query, dtype=np.float32)
    cw = np.asarray(c_weight, dtype=np.float32).reshape(H)
    qw = np.asarray(q_weight, dtype=np.float32).reshape(H)
    cqw = np.asarray(cq_weight, dtype=np.float32).reshape(H)
    bs = float(np.asarray(bias, dtype=np.float32).reshape(1)[0])

    # [:, 0:HC]=cqw cols, [:, HC:2HC]=cw cols, [:, 2HC]=bias (col j is h=j*128+p)
    wv = np.concatenate(
        [
            cqw.reshape(HC, P).T,
            cw.reshape(HC, P).T,
            np.full((P, 1), bs, np.float32),
        ],
        axis=1,
    ).astype(np.float32)
    wv = np.ascontiguousarray(wv)
    qwr = qw.reshape(HC, P).T.astype(bf16)

    in_maps = []
    for i in range(N_CORES):
        sl = slice(i * BPC, (i + 1) * BPC)
        ctx_i = context[sl]
        qry_i = query[sl]
        # ctx: [b, c, h] -> [b, p, t, h] with c = t*128+p
        ctx_s = np.ascontiguousarray(
            ctx_i.reshape(BPC, CT, P, H).transpose(0, 2, 1, 3).reshape(BPC, P, CT * H)
        ).astype(bf16)
        # ctxT: [b, h, c] -> [b, cc, p, j, cch] with h = j*128+p, c = cc*512+cch
        ctxT_s = np.ascontiguousarray(
            ctx_i.transpose(0, 2, 1)
            .reshape(BPC, HC, P, NCC, CCH)
            .transpose(0, 3, 2, 1, 4)
            .reshape(BPC, NCC, P, HC * CCH)
        ).astype(bf16)
        # qry^T: [b, h, q] -> [p, j, b, q]
        qt_s = (
            qry_i.transpose(0, 2, 1)
            .reshape(BPC, HC, P, Q)
            .transpose(2, 1, 0, 3)
            .reshape(P, HC * BPC * Q)
        ).astype(bf16)
        # qry: [b, q, h] -> [q, b, h]
        qa_s = qry_i.transpose(1, 0, 2).reshape(P, BPC * H).astype(bf16)
        qtw = np.ascontiguousarray(np.concatenate([qt_s, qwr], axis=1))
        qa_c = np.ascontiguousarray(qa_s)
        in_maps.append(
            {"ctx": ctx_s, "ctxT": ctxT_s, "qtw": qtw, "qa": qa_c, "wv": wv}
        )
    return in_maps


def kernel(context, query, c_mask, q_mask, c_weight, q_weight, cq_weight, bias):
    from concourse import bass_utils

    nc = _get_nc()
    in_maps = make_in_maps(context, query, c_weight, q_weight, cq_weight, bias)
    res = bass_utils.run_bass_kernel_spmd(nc, in_maps, core_ids=list(range(N_CORES)))

    context = np.asarray(context, dtype=np.float32)
    full = np.empty((B, C, 4 * H), dtype=np.float32)
    full[:, :, 0:H] = context
    for i in range(N_CORES):
        # device out: [b, p, t, 3h] -> [b, (t p), 3h]
        o = res.results[i]["out"].reshape(BPC, P, CT, 3 * H).transpose(0, 2, 1, 3)
        full[i * BPC : (i + 1) * BPC, :, H:] = o.reshape(BPC, C, 3 * H).astype(
            np.float32
        )
    return full
